# revision 2
# baseline (speedup 1.0000x reference)
"""Trainium2 Bass kernel for nn_MLoss_68066641707785 (topk_masking loss).

Computes, for x, y of shape [128, 43264, 5] (fp32):
    m        = (y[:,:,0] > 0.5)
    face_num = sum(m)
    scale    = 1 + 1/face_num
    diff_box = scale * sum(m * (x[:,:,1:5]-y[:,:,1:5])^2) / (face_num*4)
    bce      = -(t*log(p) + (1-t)*log(1-p)),  p = x[:,:,0], t = y[:,:,0]
    diff_c   = scale * sum(m * bce) / face_num
    diff_bg  = 0.5 * mean(-log(1-p))
    out      = diff_box + diff_c + diff_bg          (scalar fp32)

Strategy: pure data-parallel over the batch axis (16 batches per core x 8
cores). Each core streams its contiguous 27.7MB chunk of x and y through
SBUF in T tiles and reduces everything on-chip to six [128, T] partial-sum
strips (per-partition, per-tile):
    aS : sum(m*t)            bS : sum(m*(1-t))      (aS+bS = face count)
    s1 : sum(m*t*ln(p))      s2 : sum(m*(1-t)*ln(1-p))
    se : sum(m * sum_c (x_c-y_c)^2)                 (box SE, masked)
    bg : sum(ln(1-p))                               (all cells)
The host sums the 8 cores' strips in float64 and applies the final scalar
formula.  Per core: ~27.7MB HBM read, ~6KB write -> memory-bound.
"""

import numpy as np

try:
    from concourse import bacc, bass, mybir, tile
    from concourse.bass_utils import run_bass_kernel_spmd
except ImportError:  # repo not on sys.path in a fresh grading dir
    import sys

    for _p in ("/opt/trn_rl_repo", "/root/.axon_site/_ro/trn_rl_repo"):
        if _p not in sys.path:
            sys.path.insert(0, _p)
    from concourse import bacc, bass, mybir, tile
    from concourse.bass_utils import run_bass_kernel_spmd

THRESH = 0.5
ALPHA = 0.5

B, N, C = 128, 43264, 5
M = 8                      # cores
BS = B // M                # 16 batches per core
P = 128                    # SBUF partitions
W = BS * N * C // P        # 27040 fp32 per partition per core
T = 8                      # tiles per core
WT = W // T                # 3380 fp32 per partition per tile
FT = WT // C               # 676 cells per partition per tile
NSTRIP = 6

_CACHE = {}


def _build():
    f32 = mybir.dt.float32
    AF = mybir.ActivationFunctionType
    OP = mybir.AluOpType
    AX = mybir.AxisListType

    nc = bacc.Bacc("TRN2", target_bir_lowering=False, debug=False, num_devices=M)
    x_d = nc.declare_dram_parameter("x", [P, W], f32, isOutput=False)
    y_d = nc.declare_dram_parameter("y", [P, W], f32, isOutput=False)
    o_d = nc.declare_dram_parameter("o", [NSTRIP, P, T], f32, isOutput=True)
    x_ap, y_ap, o_ap = x_d[:], y_d[:], o_d[:]

    with tile.TileContext(nc) as tc:
        with tc.tile_pool(name="io", bufs=3) as io, \
             tc.tile_pool(name="mid", bufs=2) as mid, \
             tc.tile_pool(name="acc", bufs=1) as accp:
            aS = accp.tile([P, T], f32)
            bS = accp.tile([P, T], f32)
            s1S = accp.tile([P, T], f32)
            s2S = accp.tile([P, T], f32)
            seS = accp.tile([P, T], f32)
            bgS = accp.tile([P, T], f32)

            for j in range(T):
                x_t = io.tile([P, WT], f32, tag="x")
                nc.sync.dma_start(out=x_t[:], in_=x_ap[:, bass.ts(j, WT)])
                y_t = io.tile([P, WT], f32, tag="y")
                nc.sync.dma_start(out=y_t[:], in_=y_ap[:, bass.ts(j, WT)])

                xv = x_t[:].rearrange("p (f c) -> p f c", c=C)
                yv = y_t[:].rearrange("p (f c) -> p f c", c=C)
                p_ap = xv[:, :, 0]          # [P, FT] stride-5
                t_ap = yv[:, :, 0]
                xb = xv[:, :, 1:5]          # [P, FT, 4]
                yb = yv[:, :, 1:5]

                # ---- confidence channel ----
                lp = mid.tile([P, FT], f32, tag="lp")
                nc.scalar.activation(lp[:], p_ap, AF.Ln)
                lq = mid.tile([P, FT], f32, tag="lq")
                nc.scalar.activation(lq[:], p_ap, AF.Ln, bias=1.0, scale=-1.0,
                                     accum_out=bgS[:, j:j + 1])
                # a = m*t, b = m*(1-t); accums give face counts
                a = mid.tile([P, FT], f32, tag="a")
                nc.vector.scalar_tensor_tensor(
                    a[:], t_ap, THRESH, t_ap, OP.is_gt, OP.mult,
                    accum_out=aS[:, j:j + 1])
                b = mid.tile([P, FT], f32, tag="b")
                nc.vector.scalar_tensor_tensor(
                    b[:], t_ap, THRESH, a[:], OP.is_gt, OP.subtract,
                    accum_out=bS[:, j:j + 1])
                scr1 = mid.tile([P, FT], f32, tag="scr")
                nc.vector.scalar_tensor_tensor(
                    scr1[:], a[:], 1.0, lp[:], OP.mult, OP.mult,
                    accum_out=s1S[:, j:j + 1])
                scr2 = mid.tile([P, FT], f32, tag="scr")
                nc.vector.scalar_tensor_tensor(
                    scr2[:], b[:], 1.0, lq[:], OP.mult, OP.mult,
                    accum_out=s2S[:, j:j + 1])

                # ---- box channels ----
                d = mid.tile([P, 4 * FT], f32, tag="d")
                dv = d[:].rearrange("p (f c) -> p f c", c=4)
                nc.vector.tensor_sub(dv, xb, yb)
                sq = mid.tile([P, 4 * FT], f32, tag="sq")
                sqv = sq[:].rearrange("p (f c) -> p f c", c=4)
                nc.scalar.activation(sqv, dv, AF.Square)
                sec = mid.tile([P, FT], f32, tag="sec")
                nc.vector.tensor_reduce(sec[:], sqv, axis=AX.X, op=OP.add)
                scr3 = mid.tile([P, FT], f32, tag="scr")
                nc.vector.scalar_tensor_tensor(
                    scr3[:], t_ap, THRESH, sec[:], OP.is_gt, OP.mult,
                    accum_out=seS[:, j:j + 1])

            for k, strip in enumerate((aS, bS, s1S, s2S, seS, bgS)):
                nc.sync.dma_start(out=o_ap[k], in_=strip[:])

    nc.compile()
    return nc


def _get_nc():
    if "nc" not in _CACHE:
        _CACHE["nc"] = _build()
    return _CACHE["nc"]


def _in_maps(x, y):
    x = np.ascontiguousarray(np.asarray(x, dtype=np.float32))
    y = np.ascontiguousarray(np.asarray(y, dtype=np.float32))
    maps = []
    for i in range(M):
        maps.append({
            "x": x[i * BS:(i + 1) * BS].reshape(P, W),
            "y": y[i * BS:(i + 1) * BS].reshape(P, W),
        })
    return maps


def _combine(outs):
    """outs: list of M arrays [NSTRIP, P, T] -> scalar fp32 loss."""
    tot = np.zeros(NSTRIP, dtype=np.float64)
    for o in outs:
        tot += o.astype(np.float64).reshape(NSTRIP, -1).sum(axis=1)
    a_sum, b_sum, s1, s2, se, bg = tot
    face = a_sum + b_sum
    scale = 1.0 + 1.0 / face
    diff_box = scale * se / (face * 4.0)
    diff_c = scale * (-(s1 + s2)) / face
    diff_bg = ALPHA * (-bg) / (B * N)
    return np.asarray(diff_box + diff_c + diff_bg, dtype=np.float32)


def kernel(x, y, **run_kwargs):
    nc = _get_nc()
    res = run_bass_kernel_spmd(nc, _in_maps(x, y), core_ids=list(range(M)),
                               **run_kwargs)
    out = _combine([res.results[i]["o"] for i in range(M)])
    if run_kwargs:
        return out, res
    return out
